# revision 24
# baseline (speedup 1.0000x reference)
"""GatedAttentionUnit Trainium2 kernel.

Strategy (8 NeuronCores, two SPMD launches):
  Launch 1 (data-parallel projections, 1024 rows/core):
    v = silu(x@Wv+bv)            -> natural layout [row, i]
    gateT = silu(x@Wg+bg)^T      -> [i, row]
    x_ = silu(x@Wi+bi)           -> [h, row];  qT = x_*(16*gq/sqrt(I))+16*bq/sqrt(I),
                                               kT = x_*gk+bk
  Launch 2 (sequence-parallel attention, balanced causal pairing):
    core (b, j) handles query chunks j and 7-j (512 rows each) of batch b.
    Uniform SPMD: part A = 4 kv-block steps, part B = 8 steps; unused steps
    are zero-key padded (relu(0)^2 = 0 contributes nothing).
    a = relu(s)*s with s pre-scaled x16  -> a = 256*relu^2, cast to fp8 e4m3
    oT = v8.T a via fp8 DoubleRow matmuls (v8 = 64*v in e4m3, K=256/instr)
    ogT = oT * gateT/2^14 (undoes 256*64) ; y = og@Wo+bo  (bf16)

Projections/scores/output matmuls are bf16; the dominant attention*V
matmul runs in fp8 DoubleRow. Scale factors keep a/v in e4m3's normal
range (a mean ~2e-3 would otherwise hit denormals) and cost zero extra
on-chip ops: x16 folds into gamma_q, x64 into the host v->fp8 cast, and
the 2^-14 correction into the host-prepared gate.
"""
import os
import sys

for _p in ("/opt/trn_rl_repo", "/root/.axon_site/_ro/trn_rl_repo"):
    if os.path.isdir(_p) and _p not in sys.path:
        sys.path.insert(0, _p)

import numpy as np
import ml_dtypes

import concourse.bass as bass
import concourse.tile as tile
from concourse import bacc, mybir
from concourse.bass_utils import run_bass_kernel_spmd

BF16 = ml_dtypes.bfloat16
FP32 = np.float32
dt = mybir.dt

B, N, E, H, I = 2, 4096, 1024, 128, 2048
NC = 8
CH = 512            # query chunk / kv block size
RPC = 2 * CH        # rows per core
ET = E // 128       # 8 contraction tiles
IT = I // 128       # 16 i tiles
LA, LB = 4, 8       # padded kv-step counts for parts A and B
NSTEPS = LA + LB
SCALE = float(I) ** 0.5

Silu = mybir.ActivationFunctionType.Silu
Relu = mybir.ActivationFunctionType.Relu
Copy = mybir.ActivationFunctionType.Copy
MULT = mybir.AluOpType.mult
ADD = mybir.AluOpType.add
MAX = mybir.AluOpType.max
DR = mybir.MatmulPerfMode.DoubleRow
E4M3 = ml_dtypes.float8_e4m3fn
S_SCALE = 16.0       # scores pre-scale (via gamma_q); a = relu(s)^2 gets x256
V_SCALE = 64.0       # v -> fp8 pre-scale
OG_SCALE = S_SCALE * S_SCALE * V_SCALE  # gate is divided by this (2^14)

_PROG_CACHE = {}


def _build_l1(has_bv, has_bg):
    nc = bacc.Bacc("TRN2", target_bir_lowering=False, debug=False, num_devices=NC)
    xT = nc.dram_tensor("xT", [E, RPC], dt.bfloat16, kind="ExternalInput").ap()
    Wv = nc.dram_tensor("Wv", [E, I], dt.bfloat16, kind="ExternalInput").ap()
    Wg = nc.dram_tensor("Wg", [E, I], dt.bfloat16, kind="ExternalInput").ap()
    # host-packed [k_lo, e_tile*H + h] so it loads in one DMA
    Wi = nc.dram_tensor("Wi", [128, ET * H], dt.bfloat16, kind="ExternalInput").ap()
    gq = nc.dram_tensor("gq", [H, 1], dt.float32, kind="ExternalInput").ap()
    bq = nc.dram_tensor("bq", [H, 1], dt.float32, kind="ExternalInput").ap()
    gk = nc.dram_tensor("gk", [H, 1], dt.float32, kind="ExternalInput").ap()
    bk = nc.dram_tensor("bk", [H, 1], dt.float32, kind="ExternalInput").ap()
    bi = nc.dram_tensor("bi", [H, 1], dt.float32, kind="ExternalInput").ap()
    bvr = nc.dram_tensor("bvr", [1, I], dt.bfloat16, kind="ExternalInput").ap()
    bgr = nc.dram_tensor("bgr", [1, I], dt.bfloat16, kind="ExternalInput").ap()
    v_out = nc.dram_tensor("v_out", [RPC, I], dt.bfloat16, kind="ExternalOutput").ap()
    gT_out = nc.dram_tensor("gT_out", [I, RPC], dt.bfloat16, kind="ExternalOutput").ap()
    qT_out = nc.dram_tensor("qT_out", [H, RPC], dt.bfloat16, kind="ExternalOutput").ap()
    kT_out = nc.dram_tensor("kT_out", [H, RPC], dt.bfloat16, kind="ExternalOutput").ap()

    with tile.TileContext(nc) as tc:
        with (
            tc.tile_pool(name="wts", bufs=1) as wts,
            tc.tile_pool(name="io", bufs=3) as io,
            tc.tile_pool(name="ps", bufs=2, space="PSUM") as ps,
        ):
            # PE warmup spin: independent matmuls that run during the DMA
            # head so the HAM clock-gate opens before real work arrives.
            warm_sb = wts.tile([128, CH], dt.bfloat16, tag="warm", name="warm_sb")
            nc.gpsimd.memset(warm_sb[:], 0.0)
            warm_ps = ps.tile([128, CH], dt.float32, tag="vps", name="warm_ps")
            for w in range(16):
                nc.tensor.matmul(warm_ps[:], warm_sb[:, 0:128], warm_sb[:],
                                 start=True, stop=True)

            # sync issues all input loads in deadline order (~620ns each):
            # Wi + per-head vectors (xq path), xT, then Wv column blocks as
            # the v loop consumes them, Wg last (g loop runs last).
            Wi_sb = wts.tile([128, ET * H], dt.bfloat16, tag="Wi", name="Wi_sb")
            nc.sync.dma_start(Wi_sb[:], Wi[:])
            gq_sb = wts.tile([H, 1], dt.float32, tag="gq", name="gq_sb")
            nc.sync.dma_start(gq_sb[:], gq[:])
            bq_sb = wts.tile([H, 1], dt.float32, tag="bq", name="bq_sb")
            nc.sync.dma_start(bq_sb[:], bq[:])
            gk_sb = wts.tile([H, 1], dt.float32, tag="gk", name="gk_sb")
            nc.sync.dma_start(gk_sb[:], gk[:])
            bk_sb = wts.tile([H, 1], dt.float32, tag="bk", name="bk_sb")
            nc.sync.dma_start(bk_sb[:], bk[:])
            bi_sb = wts.tile([H, 1], dt.float32, tag="bi", name="bi_sb")
            nc.sync.dma_start(bi_sb[:], bi[:])
            xT_sb = []
            for e in range(ET):
                xt = wts.tile([128, RPC], dt.bfloat16, tag=f"xT{e}", name=f"xT{e}")
                nc.sync.dma_start(xt[:], xT[e * 128:(e + 1) * 128, :])
                xT_sb.append(xt)
            Wv_sb = {}
            for ib in range(I // CH):
                for e in range(ET):
                    wv = wts.tile([128, CH], dt.bfloat16, tag=f"Wv{e}_{ib}",
                                  name=f"Wv{e}_{ib}")
                    nc.sync.dma_start(
                        wv[:], Wv[e * 128:(e + 1) * 128, ib * CH:(ib + 1) * CH])
                    Wv_sb[(e, ib)] = wv
            if has_bv or has_bg:
                ones_sb = wts.tile([1, CH], dt.bfloat16, tag="ones", name="ones_sb")
                nc.gpsimd.memset(ones_sb[:], 1.0)
            if has_bv:
                bvr_sb = wts.tile([1, I], dt.bfloat16, tag="bvr", name="bvr_sb")
                nc.sync.dma_start(bvr_sb[:], bvr[:])
            if has_bg:
                bgr_sb = wts.tile([1, I], dt.bfloat16, tag="bgr", name="bgr_sb")
                nc.sync.dma_start(bgr_sb[:], bgr[:])
            Wg_sb = []
            for e in range(ET):
                wg = wts.tile([128, I], dt.bfloat16, tag=f"Wg{e}", name=f"Wg{e}")
                nc.sync.dma_start(wg[:], Wg[e * 128:(e + 1) * 128, :])
                Wg_sb.append(wg)

            # x_ -> qT, kT   [h, row]   (output DMAs issued from gpsimd)
            for rb in range(RPC // CH):
                xq_ps = ps.tile([128, CH], dt.float32, tag="xqps", name="xq_ps")
                for e in range(ET):
                    nc.tensor.matmul(
                        xq_ps[:], Wi_sb[:, e * H:e * H + H],
                        xT_sb[e][:, rb * CH:(rb + 1) * CH],
                        start=(e == 0), stop=(e == ET - 1),
                    )
                xq_sb = io.tile([128, CH], dt.float32, tag="xq", name="xq_sb")
                nc.scalar.activation(xq_sb[:], xq_ps[:], Silu, bias=bi_sb[:])
                qT_sb = io.tile([128, CH], dt.bfloat16, tag="qt", name="qT_sb")
                nc.vector.tensor_scalar(qT_sb[:], xq_sb[:], gq_sb[:], bq_sb[:], MULT, ADD)
                nc.gpsimd.dma_start(qT_out[:, rb * CH:(rb + 1) * CH], qT_sb[:])
                kT_sb = io.tile([128, CH], dt.bfloat16, tag="kt", name="kT_sb")
                nc.vector.tensor_scalar(kT_sb[:], xq_sb[:], gk_sb[:], bk_sb[:], MULT, ADD)
                nc.gpsimd.dma_start(kT_out[:, rb * CH:(rb + 1) * CH], kT_sb[:])

            # v  [row, i]  (ib outer so Wv column blocks stream just-in-time;
            # results gather into per-rt row tiles, one [128, I] write each)
            v_row = [
                io.tile([128, I], dt.bfloat16, tag=f"vrow{rt}", bufs=1,
                        name=f"v_row{rt}")
                for rt in range(RPC // 128)
            ]
            for ib in range(I // CH):
                for rt in range(RPC // 128):
                    v_ps = ps.tile([128, CH], dt.float32, tag="vps", name="v_ps")
                    for e in range(ET):
                        nc.tensor.matmul(
                            v_ps[:], xT_sb[e][:, rt * 128:(rt + 1) * 128],
                            Wv_sb[(e, ib)][:],
                            start=(e == 0), stop=(e == ET - 1 and not has_bv),
                        )
                    if has_bv:
                        nc.tensor.matmul(
                            v_ps[:], ones_sb[:, 0:128],
                            bvr_sb[:, ib * CH:(ib + 1) * CH],
                            start=False, stop=True,
                        )
                    nc.scalar.activation(
                        v_row[rt][:, ib * CH:(ib + 1) * CH], v_ps[:], Silu)
                    if ib == I // CH - 1:
                        nc.gpsimd.dma_start(
                            v_out[rt * 128:(rt + 1) * 128, :], v_row[rt][:])

            # gateT  [i, row]  (it outer; one [128, RPC] write per i tile)
            for it in range(IT):
                g_row = io.tile([128, RPC], dt.bfloat16, tag="grow", bufs=3,
                                name="g_row")
                for rb in range(RPC // CH):
                    g_ps = ps.tile([128, CH], dt.float32, tag="gps", name="g_ps")
                    for e in range(ET):
                        nc.tensor.matmul(
                            g_ps[:], Wg_sb[e][:, it * 128:(it + 1) * 128],
                            xT_sb[e][:, rb * CH:(rb + 1) * CH],
                            start=(e == 0), stop=(e == ET - 1 and not has_bg),
                        )
                    if has_bg:
                        nc.tensor.matmul(
                            g_ps[:], bgr_sb[:, it * 128:(it + 1) * 128],
                            ones_sb[:, 0:CH],
                            start=False, stop=True,
                        )
                    nc.scalar.activation(
                        g_row[:, rb * CH:(rb + 1) * CH], g_ps[:], Silu)
                nc.gpsimd.dma_start(gT_out[it * 128:(it + 1) * 128, :], g_row[:])

    nc.compile()
    return nc


def _build_l2(has_bo):
    nc = bacc.Bacc("TRN2", target_bir_lowering=False, debug=False, num_devices=NC)
    qT = nc.dram_tensor("qT", [H, RPC], dt.bfloat16, kind="ExternalInput").ap()
    # gT/Wo host-packed in i-pair-interleaved form: [pair, k_lo, half, cols]
    gT = nc.dram_tensor("gT", [IT // 2, 128, 2, RPC], dt.bfloat16, kind="ExternalInput").ap()
    Wo = nc.dram_tensor("Wo", [IT // 2, 128, 2, E], dt.bfloat16, kind="ExternalInput").ap()
    kv_k = nc.dram_tensor("kv_k", [NSTEPS, H, CH], dt.bfloat16, kind="ExternalInput").ap()
    # [step, kv-pair(256), kv_lo(128), kv_half, i] fp8: DoubleRow layout
    kv_v = nc.dram_tensor("kv_v", [NSTEPS, 2, 128, 2, I], dt.float8e4, kind="ExternalInput").ap()
    Mext = nc.dram_tensor("Mext", [128, 896], dt.float32, kind="ExternalInput").ap()
    bor = nc.dram_tensor("bor", [1, E], dt.bfloat16, kind="ExternalInput").ap()
    y = nc.dram_tensor("y", [RPC, E], dt.float32, kind="ExternalOutput").ap()

    HI = I // 2
    with tile.TileContext(nc) as tc:
        with (
            tc.tile_pool(name="wts", bufs=1) as wts,
            tc.tile_pool(name="kv", bufs=1) as kv,
            tc.tile_pool(name="kt", bufs=8) as ktp,
            tc.tile_pool(name="ap", bufs=1) as apool,
            tc.tile_pool(name="tp", bufs=3) as tp,
            tc.tile_pool(name="og", bufs=1) as ogp,
            tc.tile_pool(name="ys", bufs=3) as ysp,
            tc.tile_pool(name="sp", bufs=2, space="PSUM") as sp,
            tc.tile_pool(name="op", bufs=1, space="PSUM") as op,
            tc.tile_pool(name="yp", bufs=2, space="PSUM") as yp,
        ):
            # PE warmup spin (overlaps the resident-load DMA head)
            warm_sb = wts.tile([128, CH], dt.bfloat16, tag="warm", name="warm_sb")
            nc.gpsimd.memset(warm_sb[:], 0.0)
            warm_ps = sp.tile([128, CH], dt.float32, tag="sps", name="warm_ps")
            for w in range(16):
                nc.tensor.matmul(warm_ps[:], warm_sb[:, 0:128], warm_sb[:],
                                 start=True, stop=True)

            qT_sb = wts.tile([H, RPC], dt.bfloat16, tag="qT", name="qT_sb")
            nc.sync.dma_start(qT_sb[:], qT[:])
            Mext_sb = wts.tile([128, 896], dt.float32, tag="Mext", name="Mext_sb")
            nc.sync.dma_start(Mext_sb[:], Mext[:])
            if has_bo:
                ones_sb = wts.tile([1, 128], dt.bfloat16, tag="ones", name="ones_sb")
                nc.gpsimd.memset(ones_sb[:], 1.0)
                bor_sb = wts.tile([1, E], dt.bfloat16, tag="bor", name="bor_sb")
                nc.sync.dma_start(bor_sb[:], bor[:])

            # ---- sync issue stream in consumption-deadline order ----
            kt_sb = {}

            def issue_kt(part, L, sbase):
                for t in range(L):
                    kt = ktp.tile([H, CH], dt.bfloat16, tag="kt",
                                  name=f"kt{part}_{t}")
                    nc.sync.dma_start(kt[:], kv_k[sbase + t])
                    kt_sb[(part, t)] = kt

            vt_sb = {}

            def issue_vt(part, sbase, trange, hf):
                for t in trange:
                    for p in range(2):
                        vt = kv.tile([128, 2, HI], dt.float8e4,
                                     tag=f"vt{t}_{p}_{hf}",
                                     name=f"vt{part}_{t}_{p}_{hf}")
                        nc.sync.dma_start(
                            vt[:],
                            kv_v[sbase + t, p, :, :, hf * HI:(hf + 1) * HI])
                        vt_sb[(part, t, p, hf)] = vt

            gT_sb = [None] * (IT // 2)
            Wo_sb = [None] * (IT // 2)

            def issue_gt(pairs):
                for tp2 in pairs:
                    g = wts.tile([128, 2, RPC], dt.bfloat16, tag=f"gT{tp2}",
                                 name=f"gT{tp2}")
                    nc.sync.dma_start(g[:], gT[tp2])
                    gT_sb[tp2] = g

            def issue_wo(pairs):
                for tp2 in pairs:
                    w = wts.tile([128, 2, E], dt.bfloat16, tag=f"Wo{tp2}",
                                 name=f"Wo{tp2}")
                    nc.sync.dma_start(w[:], Wo[tp2])
                    Wo_sb[tp2] = w

            issue_kt(0, LA, 0)
            issue_vt(0, 0, range(LA), 0)
            issue_kt(1, LB, LA)
            issue_gt([0, 1])
            issue_vt(0, 0, range(LA), 1)
            issue_gt([2, 3, 4, 5, 6, 7])
            issue_wo(range(IT // 2))
            # part B v: fresh tags (t>=LA) first; reused tags last so the
            # in-order sync stream never stalls ahead of free work
            issue_vt(1, LA, range(LA, LB), 0)
            issue_vt(1, LA, range(LA, LB), 1)
            issue_vt(1, LA, range(LA), 0)
            issue_vt(1, LA, range(LA), 1)

            # ---- compute phases ----
            a_sb = {}

            def emit_scores_quad(part, t, p):
                # 2 score matmuls + relu + square for kv pair (t, p).
                # start=stop singles amid open o_ps groups: separate banks.
                rcol = slice(part * CH, (part + 1) * CH)
                kt = kt_sb[(part, t)]
                a = apool.tile([128, 2, CH], dt.float8e4,
                               bufs=2 if t < LA else 1,
                               tag=f"a{t}_{p}", name=f"a{part}_{t}_{p}")
                for h in range(2):
                    mt = 2 * p + h
                    s_ps = sp.tile([128, CH], dt.float32, tag="sps",
                                   name="s_ps")
                    nc.tensor.matmul(
                        s_ps[:], kt[:, mt * 128:(mt + 1) * 128],
                        qT_sb[:, rcol], start=True, stop=True,
                        skip_group_check=True,
                    )
                    if t == 0:  # diagonal block: causal mask
                        off = 384 - 128 * mt
                        nc.vector.tensor_mul(
                            s_ps[:], s_ps[:], Mext_sb[:, off:off + CH])
                    t_sb = tp.tile([128, CH], dt.bfloat16, tag="tsb",
                                   name="t_sb")
                    nc.scalar.activation(t_sb[:], s_ps[:], Relu)
                    nc.vector.tensor_mul(a[:, h, :], t_sb[:], t_sb[:])
                a_sb[(part, t, p)] = a

            og_sb = {}

            def emit_pv_iq(part, iq, torder, after_group=None):
                # one PSUM accumulation pass (4 i-columns) over all kv pairs;
                # after_group(idx) lets score quads ride the PV stream
                rcol = slice(part * CH, (part + 1) * CH)
                pairs = [(t, p) for t in torder for p in range(2)]
                hf, ci = iq // 2, (iq % 2) * CH
                o_ps = [
                    op.tile([128, CH], dt.float32, tag=f"o{j}", name=f"o{j}")
                    for j in range(4)
                ]
                for idx, (t, p) in enumerate(pairs):
                    for j in range(4):
                        col = ci + j * 128
                        nc.tensor.matmul(
                            o_ps[j][:],
                            vt_sb[(part, t, p, hf)][:, :, col:col + 128],
                            a_sb[(part, t, p)][:],
                            start=(idx == 0),
                            stop=(idx == len(pairs) - 1),
                            perf_mode=DR,
                        )
                    if after_group:
                        after_group(idx)
                for j in range(4):
                    it = iq * 4 + j
                    og = ogp.tile([128, CH], dt.bfloat16,
                                  tag=f"og{it}", name=f"og{part}_{it}")
                    nc.vector.tensor_mul(
                        og[:], o_ps[j][:], gT_sb[it // 2][:, it % 2, rcol])
                    og_sb[(part, it)] = og

            def emit_wo(part, inject=None):
                inj = list(inject) if inject else []
                for rt in range(4):
                    for eb in range(E // CH):
                        if inj:
                            inj.pop(0)()
                        y_ps = yp.tile([128, CH], dt.float32, tag="yps",
                                       name="y_ps")
                        for it in range(IT):
                            nc.tensor.matmul(
                                y_ps[:],
                                og_sb[(part, it)][:, rt * 128:(rt + 1) * 128],
                                Wo_sb[it // 2][:, it % 2,
                                               eb * CH:(eb + 1) * CH],
                                start=(it == 0),
                                stop=(it == IT - 1 and not has_bo),
                            )
                        if has_bo:
                            nc.tensor.matmul(
                                y_ps[:], ones_sb[:, 0:128],
                                bor_sb[:, eb * CH:(eb + 1) * CH],
                                start=False, stop=True,
                            )
                        y_sb = ysp.tile([128, CH], dt.float32, tag="ysb",
                                        bufs=6, name="y_sb")
                        nc.scalar.activation(y_sb[:], y_ps[:], Copy)
                        nc.gpsimd.dma_start(
                            y[part * CH + rt * 128:part * CH + (rt + 1) * 128,
                              eb * CH:(eb + 1) * CH], y_sb[:])

            # part-A scores lockstep with PV-A iq0 (2-quad lead) so the
            # tensor engine never throttles to the scalar relu rate
            aq = [(t, p) for t in range(LA) for p in range(2)]
            emit_scores_quad(0, *aq[0])
            emit_scores_quad(0, *aq[1])

            def il0(idx):
                if idx + 2 < len(aq):
                    emit_scores_quad(0, *aq[idx + 2])

            emit_pv_iq(0, 0, list(range(LA)), after_group=il0)
            # part-B score quads ride inside PV-A iq1-3 and Wo-A
            inject = [
                (lambda t=t, p=p: emit_scores_quad(1, t, p))
                for t in range(LB) for p in range(2)
            ]

            def ilb(idx):
                if idx % 2 == 1 and inject:
                    inject.pop(0)()

            for iq in range(1, 4):
                emit_pv_iq(0, iq, list(range(LA)), after_group=ilb)
            emit_wo(0, inject=inject)
            # consume preloaded fresh (t>=LA) v tiles first
            for iq in range(4):
                emit_pv_iq(1, iq, list(range(LA, LB)) + list(range(LA)))
            emit_wo(1)

    nc.compile()
    return nc


def _get_prog(which, *flags):
    key = (which,) + flags
    if key not in _PROG_CACHE:
        _PROG_CACHE[key] = _build_l1(*flags) if which == 1 else _build_l2(*flags)
    return _PROG_CACHE[key]


# core -> (batch, chunkA, chunkB): balanced causal pairing
_ASSIGN = [(b, j, 7 - j) for b in range(B) for j in range(4)]


def kernel(x, Wv, bv, Wg, bg, Wi, bi, gamma_q, beta_q, gamma_k, beta_k, Wo, bo):
    x = np.asarray(x, FP32)
    Wv = np.asarray(Wv, FP32); bv = np.asarray(bv, FP32)
    Wg = np.asarray(Wg, FP32); bg = np.asarray(bg, FP32)
    Wi = np.asarray(Wi, FP32); bi = np.asarray(bi, FP32)
    gamma_q = np.asarray(gamma_q, FP32); beta_q = np.asarray(beta_q, FP32)
    gamma_k = np.asarray(gamma_k, FP32); beta_k = np.asarray(beta_k, FP32)
    Wo = np.asarray(Wo, FP32); bo = np.asarray(bo, FP32)

    has_bv = bool(np.any(bv)); has_bg = bool(np.any(bg)); has_bo = bool(np.any(bo))
    nc1 = _get_prog(1, has_bv, has_bg)
    nc2 = _get_prog(2, has_bo)

    Wv_b = Wv.astype(BF16); Wg_b = Wg.astype(BF16)
    # Wi packed [k_lo, e_tile*H + h] for a single L1 DMA
    Wi_b = np.ascontiguousarray(
        Wi.astype(BF16).reshape(ET, 128, H).transpose(1, 0, 2).reshape(128, ET * H))
    # Wo packed [i_pair, k_lo, half, e] for DoubleRow-style paired loads
    Wo_b = np.ascontiguousarray(
        Wo.astype(BF16).reshape(IT // 2, 2, 128, E).transpose(0, 2, 1, 3))
    gq = (gamma_q * (S_SCALE / SCALE)).astype(FP32).reshape(H, 1)
    bq = (beta_q * (S_SCALE / SCALE)).astype(FP32).reshape(H, 1)
    gk = gamma_k.reshape(H, 1).copy(); bk = beta_k.reshape(H, 1).copy()
    bi_c = bi.reshape(H, 1).copy()
    bvr = bv.reshape(1, I).astype(BF16); bgr = bg.reshape(1, I).astype(BF16)
    bor = bo.reshape(1, E).astype(BF16)

    xg = x.reshape(B, N // CH, CH, E)
    in1 = []
    for (b, cA, cB) in _ASSIGN:
        xc = np.concatenate([xg[b, cA], xg[b, cB]], axis=0)  # [RPC, E]
        in1.append({
            "xT": np.ascontiguousarray(xc.T).astype(BF16),
            "Wv": Wv_b, "Wg": Wg_b, "Wi": Wi_b,
            "gq": gq, "bq": bq, "gk": gk, "bk": bk, "bi": bi_c,
            "bvr": bvr, "bgr": bgr,
        })
    res1 = run_bass_kernel_spmd(nc1, in1, core_ids=list(range(NC)))

    vfull = np.zeros((B, N // CH, CH, I), E4M3)
    kTfull = np.zeros((B, H, N), BF16)
    for c, (b, cA, cB) in enumerate(_ASSIGN):
        r = res1.results[c]
        v8 = (r["v_out"].astype(FP32) * V_SCALE).astype(E4M3)
        vfull[b, cA] = v8[:CH]
        vfull[b, cB] = v8[CH:]
        kTfull[b][:, cA * CH:(cA + 1) * CH] = r["kT_out"][:, :CH]
        kTfull[b][:, cB * CH:(cB + 1) * CH] = r["kT_out"][:, CH:]

    # extended causal mask: Mext[p, u] = 1 iff u >= p + 384
    Mext = (np.arange(896)[None, :] >= (np.arange(128)[:, None] + 384)).astype(FP32)

    in2 = []
    for c, (b, cA, cB) in enumerate(_ASSIGN):
        r = res1.results[c]
        kvk = np.zeros((NSTEPS, H, CH), BF16)
        kvv = np.zeros((NSTEPS, CH, I), E4M3)
        for base, cq in ((0, cA), (LA, cB)):
            kvk[base] = kTfull[b][:, cq * CH:(cq + 1) * CH]
            kvv[base] = vfull[b, cq]
            for idx in range(cq):
                kvk[base + 1 + idx] = kTfull[b][:, idx * CH:(idx + 1) * CH]
                kvv[base + 1 + idx] = vfull[b, idx]
        # [t, pair, kv_lo, kv_half, i] for DoubleRow lhsT tiles
        kvv2 = np.ascontiguousarray(
            kvv.reshape(NSTEPS, 2, 2, 128, I).transpose(0, 1, 3, 2, 4))
        gT_s = (r["gT_out"].astype(FP32) * (1.0 / OG_SCALE)).astype(BF16)
        gT_p = np.ascontiguousarray(
            gT_s.reshape(IT // 2, 2, 128, RPC).transpose(0, 2, 1, 3))
        in2.append({
            "qT": r["qT_out"], "gT": gT_p, "Wo": Wo_b,
            "kv_k": kvk, "kv_v": kvv2, "Mext": Mext, "bor": bor,
        })
    res2 = run_bass_kernel_spmd(nc2, in2, core_ids=list(range(NC)))

    out = np.zeros((B, N // CH, CH, E), FP32)
    for c, (b, cA, cB) in enumerate(_ASSIGN):
        yy = res2.results[c]["y"]
        out[b, cA] = yy[:CH]
        out[b, cB] = yy[CH:]
    return out.reshape(B, N, E)



# revision 26
# speedup vs baseline: 1.0154x; 1.0154x over previous
"""GatedAttentionUnit Trainium2 kernel.

Strategy (8 NeuronCores, two SPMD launches):
  Launch 1 (data-parallel projections, 1024 rows/core):
    v = silu(x@Wv+bv)            -> natural layout [row, i]
    gateT = silu(x@Wg+bg)^T      -> [i, row]
    x_ = silu(x@Wi+bi)           -> [h, row];  qT = x_*(16*gq/sqrt(I))+16*bq/sqrt(I),
                                               kT = x_*gk+bk
  Launch 2 (sequence-parallel attention, balanced causal pairing):
    core (b, j) handles query chunks j and 7-j (512 rows each) of batch b.
    Uniform SPMD: part A = 4 kv-block steps, part B = 8 steps; unused steps
    are zero-key padded (relu(0)^2 = 0 contributes nothing).
    a = relu(s)*s with s pre-scaled x16  -> a = 256*relu^2, cast to fp8 e4m3
    oT = v8.T a via fp8 DoubleRow matmuls (v8 = 64*v in e4m3, K=256/instr)
    ogT = oT * gateT/2^14 (undoes 256*64) ; y = og@Wo+bo  (bf16)

Projections/scores/output matmuls are bf16; the dominant attention*V
matmul runs in fp8 DoubleRow. Scale factors keep a/v in e4m3's normal
range (a mean ~2e-3 would otherwise hit denormals) and cost zero extra
on-chip ops: x16 folds into gamma_q, x64 into the host v->fp8 cast, and
the 2^-14 correction into the host-prepared gate.
"""
import os
import sys

for _p in ("/opt/trn_rl_repo", "/root/.axon_site/_ro/trn_rl_repo"):
    if os.path.isdir(_p) and _p not in sys.path:
        sys.path.insert(0, _p)

import numpy as np
import ml_dtypes

import concourse.bass as bass
import concourse.tile as tile
from concourse import bacc, mybir
from concourse.bass_utils import run_bass_kernel_spmd

BF16 = ml_dtypes.bfloat16
FP32 = np.float32
dt = mybir.dt

B, N, E, H, I = 2, 4096, 1024, 128, 2048
NC = 8
CH = 512            # query chunk / kv block size
RPC = 2 * CH        # rows per core
ET = E // 128       # 8 contraction tiles
IT = I // 128       # 16 i tiles
LA, LB = 4, 8       # padded kv-step counts for parts A and B
NSTEPS = LA + LB
SCALE = float(I) ** 0.5

Silu = mybir.ActivationFunctionType.Silu
Relu = mybir.ActivationFunctionType.Relu
Copy = mybir.ActivationFunctionType.Copy
MULT = mybir.AluOpType.mult
ADD = mybir.AluOpType.add
MAX = mybir.AluOpType.max
DR = mybir.MatmulPerfMode.DoubleRow
E4M3 = ml_dtypes.float8_e4m3fn
S_SCALE = 16.0       # scores pre-scale (via gamma_q); a = relu(s)^2 gets x256
V_SCALE = 64.0       # v -> fp8 pre-scale
OG_SCALE = S_SCALE * S_SCALE * V_SCALE  # gate is divided by this (2^14)

_PROG_CACHE = {}


def _build_l1(has_bv, has_bg):
    nc = bacc.Bacc("TRN2", target_bir_lowering=False, debug=False, num_devices=NC)
    xT = nc.dram_tensor("xT", [E, RPC], dt.bfloat16, kind="ExternalInput").ap()
    Wv = nc.dram_tensor("Wv", [E, I], dt.bfloat16, kind="ExternalInput").ap()
    Wg = nc.dram_tensor("Wg", [E, I], dt.bfloat16, kind="ExternalInput").ap()
    # host-packed [k_lo, e_tile*H + h] so it loads in one DMA
    Wi = nc.dram_tensor("Wi", [128, ET * H], dt.bfloat16, kind="ExternalInput").ap()
    gq = nc.dram_tensor("gq", [H, 1], dt.float32, kind="ExternalInput").ap()
    bq = nc.dram_tensor("bq", [H, 1], dt.float32, kind="ExternalInput").ap()
    gk = nc.dram_tensor("gk", [H, 1], dt.float32, kind="ExternalInput").ap()
    bk = nc.dram_tensor("bk", [H, 1], dt.float32, kind="ExternalInput").ap()
    bi = nc.dram_tensor("bi", [H, 1], dt.float32, kind="ExternalInput").ap()
    bvr = nc.dram_tensor("bvr", [1, I], dt.bfloat16, kind="ExternalInput").ap()
    bgr = nc.dram_tensor("bgr", [1, I], dt.bfloat16, kind="ExternalInput").ap()
    v_out = nc.dram_tensor("v_out", [RPC, I], dt.bfloat16, kind="ExternalOutput").ap()
    gT_out = nc.dram_tensor("gT_out", [I, RPC], dt.bfloat16, kind="ExternalOutput").ap()
    qT_out = nc.dram_tensor("qT_out", [H, RPC], dt.bfloat16, kind="ExternalOutput").ap()
    kT_out = nc.dram_tensor("kT_out", [H, RPC], dt.bfloat16, kind="ExternalOutput").ap()

    with tile.TileContext(nc) as tc:
        with (
            tc.tile_pool(name="wts", bufs=1) as wts,
            tc.tile_pool(name="io", bufs=3) as io,
            tc.tile_pool(name="ps", bufs=2, space="PSUM") as ps,
        ):
            # PE warmup spin: independent matmuls that run during the DMA
            # head so the HAM clock-gate opens before real work arrives.
            warm_sb = wts.tile([128, CH], dt.bfloat16, tag="warm", name="warm_sb")
            nc.gpsimd.memset(warm_sb[:], 0.0)
            warm_ps = ps.tile([128, CH], dt.float32, tag="vps", name="warm_ps")
            for w in range(16):
                nc.tensor.matmul(warm_ps[:], warm_sb[:, 0:128], warm_sb[:],
                                 start=True, stop=True)

            # sync issues all input loads in deadline order (~620ns each):
            # Wi + per-head vectors (xq path), xT, then Wv column blocks as
            # the v loop consumes them, Wg last (g loop runs last).
            Wi_sb = wts.tile([128, ET * H], dt.bfloat16, tag="Wi", name="Wi_sb")
            nc.sync.dma_start(Wi_sb[:], Wi[:])
            gq_sb = wts.tile([H, 1], dt.float32, tag="gq", name="gq_sb")
            nc.sync.dma_start(gq_sb[:], gq[:])
            bq_sb = wts.tile([H, 1], dt.float32, tag="bq", name="bq_sb")
            nc.sync.dma_start(bq_sb[:], bq[:])
            gk_sb = wts.tile([H, 1], dt.float32, tag="gk", name="gk_sb")
            nc.sync.dma_start(gk_sb[:], gk[:])
            bk_sb = wts.tile([H, 1], dt.float32, tag="bk", name="bk_sb")
            nc.sync.dma_start(bk_sb[:], bk[:])
            bi_sb = wts.tile([H, 1], dt.float32, tag="bi", name="bi_sb")
            nc.sync.dma_start(bi_sb[:], bi[:])
            xT_sb = []
            for e in range(ET):
                xt = wts.tile([128, RPC], dt.bfloat16, tag=f"xT{e}", name=f"xT{e}")
                nc.sync.dma_start(xt[:], xT[e * 128:(e + 1) * 128, :])
                xT_sb.append(xt)
            Wv_sb = {}
            for ib in range(I // CH):
                for e in range(ET):
                    wv = wts.tile([128, CH], dt.bfloat16, tag=f"Wv{e}_{ib}",
                                  name=f"Wv{e}_{ib}")
                    nc.sync.dma_start(
                        wv[:], Wv[e * 128:(e + 1) * 128, ib * CH:(ib + 1) * CH])
                    Wv_sb[(e, ib)] = wv
            if has_bv or has_bg:
                ones_sb = wts.tile([1, CH], dt.bfloat16, tag="ones", name="ones_sb")
                nc.gpsimd.memset(ones_sb[:], 1.0)
            if has_bv:
                bvr_sb = wts.tile([1, I], dt.bfloat16, tag="bvr", name="bvr_sb")
                nc.sync.dma_start(bvr_sb[:], bvr[:])
            if has_bg:
                bgr_sb = wts.tile([1, I], dt.bfloat16, tag="bgr", name="bgr_sb")
                nc.sync.dma_start(bgr_sb[:], bgr[:])
            Wg_sb = []
            for e in range(ET):
                wg = wts.tile([128, I], dt.bfloat16, tag=f"Wg{e}", name=f"Wg{e}")
                nc.sync.dma_start(wg[:], Wg[e * 128:(e + 1) * 128, :])
                Wg_sb.append(wg)

            # x_ -> qT, kT   [h, row]   (output DMAs issued from gpsimd)
            for rb in range(RPC // CH):
                xq_ps = ps.tile([128, CH], dt.float32, tag="xqps", name="xq_ps")
                for e in range(ET):
                    nc.tensor.matmul(
                        xq_ps[:], Wi_sb[:, e * H:e * H + H],
                        xT_sb[e][:, rb * CH:(rb + 1) * CH],
                        start=(e == 0), stop=(e == ET - 1),
                    )
                xq_sb = io.tile([128, CH], dt.float32, tag="xq", name="xq_sb")
                nc.scalar.activation(xq_sb[:], xq_ps[:], Silu, bias=bi_sb[:])
                qT_sb = io.tile([128, CH], dt.bfloat16, tag="qt", name="qT_sb")
                nc.vector.tensor_scalar(qT_sb[:], xq_sb[:], gq_sb[:], bq_sb[:], MULT, ADD)
                nc.gpsimd.dma_start(qT_out[:, rb * CH:(rb + 1) * CH], qT_sb[:])
                kT_sb = io.tile([128, CH], dt.bfloat16, tag="kt", name="kT_sb")
                nc.vector.tensor_scalar(kT_sb[:], xq_sb[:], gk_sb[:], bk_sb[:], MULT, ADD)
                nc.gpsimd.dma_start(kT_out[:, rb * CH:(rb + 1) * CH], kT_sb[:])

            # v  [row, i]  (ib outer so Wv column blocks stream just-in-time;
            # results gather into per-rt row tiles, one [128, I] write each)
            v_row = [
                io.tile([128, I], dt.bfloat16, tag=f"vrow{rt}", bufs=1,
                        name=f"v_row{rt}")
                for rt in range(RPC // 128)
            ]
            for ib in range(I // CH):
                for rt in range(RPC // 128):
                    v_ps = ps.tile([128, CH], dt.float32, tag="vps", name="v_ps")
                    for e in range(ET):
                        nc.tensor.matmul(
                            v_ps[:], xT_sb[e][:, rt * 128:(rt + 1) * 128],
                            Wv_sb[(e, ib)][:],
                            start=(e == 0), stop=(e == ET - 1 and not has_bv),
                        )
                    if has_bv:
                        nc.tensor.matmul(
                            v_ps[:], ones_sb[:, 0:128],
                            bvr_sb[:, ib * CH:(ib + 1) * CH],
                            start=False, stop=True,
                        )
                    nc.scalar.activation(
                        v_row[rt][:, ib * CH:(ib + 1) * CH], v_ps[:], Silu)
                    if ib == I // CH - 1:
                        nc.gpsimd.dma_start(
                            v_out[rt * 128:(rt + 1) * 128, :], v_row[rt][:])

            # gateT  [i, row]  (it outer; one [128, RPC] write per i tile)
            for it in range(IT):
                g_row = io.tile([128, RPC], dt.bfloat16, tag="grow", bufs=3,
                                name="g_row")
                for rb in range(RPC // CH):
                    g_ps = ps.tile([128, CH], dt.float32, tag="gps", name="g_ps")
                    for e in range(ET):
                        nc.tensor.matmul(
                            g_ps[:], Wg_sb[e][:, it * 128:(it + 1) * 128],
                            xT_sb[e][:, rb * CH:(rb + 1) * CH],
                            start=(e == 0), stop=(e == ET - 1 and not has_bg),
                        )
                    if has_bg:
                        nc.tensor.matmul(
                            g_ps[:], bgr_sb[:, it * 128:(it + 1) * 128],
                            ones_sb[:, 0:CH],
                            start=False, stop=True,
                        )
                    nc.scalar.activation(
                        g_row[:, rb * CH:(rb + 1) * CH], g_ps[:], Silu)
                nc.gpsimd.dma_start(gT_out[it * 128:(it + 1) * 128, :], g_row[:])

    nc.compile()
    return nc


def _build_l2(has_bo):
    nc = bacc.Bacc("TRN2", target_bir_lowering=False, debug=False, num_devices=NC)
    qT = nc.dram_tensor("qT", [H, RPC], dt.bfloat16, kind="ExternalInput").ap()
    # gT/Wo host-packed in i-pair-interleaved form: [pair, k_lo, half, cols]
    gT = nc.dram_tensor("gT", [IT // 2, 128, 2, RPC], dt.bfloat16, kind="ExternalInput").ap()
    Wo = nc.dram_tensor("Wo", [IT // 2, 128, 2, E], dt.bfloat16, kind="ExternalInput").ap()
    kv_k = nc.dram_tensor("kv_k", [NSTEPS, H, CH], dt.bfloat16, kind="ExternalInput").ap()
    # [step, kv-pair(256), kv_lo(128), kv_half, i] fp8: DoubleRow layout
    kv_v = nc.dram_tensor("kv_v", [NSTEPS, 2, 128, 2, I], dt.float8e4, kind="ExternalInput").ap()
    Mext = nc.dram_tensor("Mext", [128, 896], dt.float32, kind="ExternalInput").ap()
    bor = nc.dram_tensor("bor", [1, E], dt.bfloat16, kind="ExternalInput").ap()
    y = nc.dram_tensor("y", [RPC, E], dt.float32, kind="ExternalOutput").ap()

    HI = I // 2
    with tile.TileContext(nc) as tc:
        with (
            tc.tile_pool(name="wts", bufs=1) as wts,
            tc.tile_pool(name="kv", bufs=1) as kv,
            tc.tile_pool(name="kt", bufs=8) as ktp,
            tc.tile_pool(name="ap", bufs=1) as apool,
            tc.tile_pool(name="tp", bufs=3) as tp,
            tc.tile_pool(name="og", bufs=1) as ogp,
            tc.tile_pool(name="ys", bufs=3) as ysp,
            tc.tile_pool(name="sp", bufs=2, space="PSUM") as sp,
            tc.tile_pool(name="op", bufs=1, space="PSUM") as op,
            tc.tile_pool(name="yp", bufs=2, space="PSUM") as yp,
        ):
            # PE warmup spin (overlaps the resident-load DMA head)
            warm_sb = wts.tile([128, CH], dt.bfloat16, tag="warm", name="warm_sb")
            nc.gpsimd.memset(warm_sb[:], 0.0)
            warm_ps = sp.tile([128, CH], dt.float32, tag="sps", name="warm_ps")
            for w in range(8):
                nc.tensor.matmul(warm_ps[:], warm_sb[:, 0:128], warm_sb[:],
                                 start=True, stop=True)

            qT_sb = wts.tile([H, RPC], dt.bfloat16, tag="qT", name="qT_sb")
            nc.sync.dma_start(qT_sb[:], qT[:])
            Mext_sb = wts.tile([128, 896], dt.float32, tag="Mext", name="Mext_sb")
            nc.sync.dma_start(Mext_sb[:], Mext[:])
            if has_bo:
                ones_sb = wts.tile([1, 128], dt.bfloat16, tag="ones", name="ones_sb")
                nc.gpsimd.memset(ones_sb[:], 1.0)
                bor_sb = wts.tile([1, E], dt.bfloat16, tag="bor", name="bor_sb")
                nc.sync.dma_start(bor_sb[:], bor[:])

            # ---- sync issue stream in consumption-deadline order ----
            kt_sb = {}

            def issue_kt(part, L, sbase):
                for t in range(L):
                    kt = ktp.tile([H, CH], dt.bfloat16, tag="kt",
                                  name=f"kt{part}_{t}")
                    nc.sync.dma_start(kt[:], kv_k[sbase + t])
                    kt_sb[(part, t)] = kt

            vt_sb = {}

            def issue_vt(part, sbase, trange, hf):
                for t in trange:
                    for p in range(2):
                        vt = kv.tile([128, 2, HI], dt.float8e4,
                                     tag=f"vt{t}_{p}_{hf}",
                                     name=f"vt{part}_{t}_{p}_{hf}")
                        nc.sync.dma_start(
                            vt[:],
                            kv_v[sbase + t, p, :, :, hf * HI:(hf + 1) * HI])
                        vt_sb[(part, t, p, hf)] = vt

            gT_sb = [None] * (IT // 2)
            Wo_sb = [None] * (IT // 2)

            def issue_gt(pairs):
                for tp2 in pairs:
                    g = wts.tile([128, 2, RPC], dt.bfloat16, tag=f"gT{tp2}",
                                 name=f"gT{tp2}")
                    nc.sync.dma_start(g[:], gT[tp2])
                    gT_sb[tp2] = g

            def issue_wo(pairs):
                for tp2 in pairs:
                    w = wts.tile([128, 2, E], dt.bfloat16, tag=f"Wo{tp2}",
                                 name=f"Wo{tp2}")
                    nc.sync.dma_start(w[:], Wo[tp2])
                    Wo_sb[tp2] = w

            issue_kt(0, LA, 0)
            issue_vt(0, 0, range(LA), 0)
            issue_kt(1, LB, LA)
            issue_gt([0, 1])
            issue_vt(0, 0, range(LA), 1)
            issue_gt([2, 3, 4, 5, 6, 7])
            issue_wo(range(IT // 2))
            # part B v: fresh tags (t>=LA) first; reused tags last so the
            # in-order sync stream never stalls ahead of free work
            issue_vt(1, LA, range(LA, LB), 0)
            issue_vt(1, LA, range(LA, LB), 1)
            issue_vt(1, LA, range(LA), 0)
            issue_vt(1, LA, range(LA), 1)

            # ---- compute phases ----
            a_sb = {}

            def emit_scores_quad(part, t, p):
                # 2 score matmuls + relu + square for kv pair (t, p).
                # start=stop singles amid open o_ps groups: separate banks.
                rcol = slice(part * CH, (part + 1) * CH)
                kt = kt_sb[(part, t)]
                a = apool.tile([128, 2, CH], dt.float8e4,
                               bufs=2 if t < LA else 1,
                               tag=f"a{t}_{p}", name=f"a{part}_{t}_{p}")
                for h in range(2):
                    mt = 2 * p + h
                    s_ps = sp.tile([128, CH], dt.float32, tag="sps",
                                   name="s_ps")
                    nc.tensor.matmul(
                        s_ps[:], kt[:, mt * 128:(mt + 1) * 128],
                        qT_sb[:, rcol], start=True, stop=True,
                        skip_group_check=True,
                    )
                    if t == 0:  # diagonal block: causal mask
                        off = 384 - 128 * mt
                        nc.vector.tensor_mul(
                            s_ps[:], s_ps[:], Mext_sb[:, off:off + CH])
                    t_sb = tp.tile([128, CH], dt.bfloat16, tag="tsb",
                                   name="t_sb")
                    nc.scalar.activation(t_sb[:], s_ps[:], Relu)
                    nc.vector.tensor_mul(a[:, h, :], t_sb[:], t_sb[:])
                a_sb[(part, t, p)] = a

            og_sb = {}

            def emit_pv_iq(part, iq, torder, after_group=None):
                # one PSUM accumulation pass (4 i-columns) over all kv pairs;
                # after_group(idx) lets score quads ride the PV stream
                rcol = slice(part * CH, (part + 1) * CH)
                pairs = [(t, p) for t in torder for p in range(2)]
                hf, ci = iq // 2, (iq % 2) * CH
                o_ps = [
                    op.tile([128, CH], dt.float32, tag=f"o{j}", name=f"o{j}")
                    for j in range(4)
                ]
                for idx, (t, p) in enumerate(pairs):
                    for j in range(4):
                        col = ci + j * 128
                        nc.tensor.matmul(
                            o_ps[j][:],
                            vt_sb[(part, t, p, hf)][:, :, col:col + 128],
                            a_sb[(part, t, p)][:],
                            start=(idx == 0),
                            stop=(idx == len(pairs) - 1),
                            perf_mode=DR,
                        )
                    if after_group:
                        after_group(idx)
                for j in range(4):
                    it = iq * 4 + j
                    og = ogp.tile([128, CH], dt.bfloat16,
                                  tag=f"og{it}", name=f"og{part}_{it}")
                    nc.vector.tensor_mul(
                        og[:], o_ps[j][:], gT_sb[it // 2][:, it % 2, rcol])
                    og_sb[(part, it)] = og

            def emit_wo(part, inject=None):
                inj = list(inject) if inject else []
                for rt in range(4):
                    for eb in range(E // CH):
                        if inj:
                            inj.pop(0)()
                        y_ps = yp.tile([128, CH], dt.float32, tag="yps",
                                       name="y_ps")
                        for it in range(IT):
                            nc.tensor.matmul(
                                y_ps[:],
                                og_sb[(part, it)][:, rt * 128:(rt + 1) * 128],
                                Wo_sb[it // 2][:, it % 2,
                                               eb * CH:(eb + 1) * CH],
                                start=(it == 0),
                                stop=(it == IT - 1 and not has_bo),
                            )
                        if has_bo:
                            nc.tensor.matmul(
                                y_ps[:], ones_sb[:, 0:128],
                                bor_sb[:, eb * CH:(eb + 1) * CH],
                                start=False, stop=True,
                            )
                        y_sb = ysp.tile([128, CH], dt.float32, tag="ysb",
                                        bufs=6, name="y_sb")
                        nc.scalar.activation(y_sb[:], y_ps[:], Copy)
                        nc.gpsimd.dma_start(
                            y[part * CH + rt * 128:part * CH + (rt + 1) * 128,
                              eb * CH:(eb + 1) * CH], y_sb[:])

            # part-A scores lockstep with PV-A iq0 (2-quad lead) so the
            # tensor engine never throttles to the scalar relu rate
            aq = [(t, p) for t in range(LA) for p in range(2)]
            emit_scores_quad(0, *aq[0])
            emit_scores_quad(0, *aq[1])
            emit_scores_quad(0, *aq[2])

            def il0(idx):
                if idx + 3 < len(aq):
                    emit_scores_quad(0, *aq[idx + 3])

            emit_pv_iq(0, 0, list(range(LA)), after_group=il0)
            # part-B score quads ride inside PV-A iq1-3 and Wo-A
            inject = [
                (lambda t=t, p=p: emit_scores_quad(1, t, p))
                for t in range(LB) for p in range(2)
            ]

            def ilb(idx):
                if idx % 2 == 1 and inject:
                    inject.pop(0)()

            for iq in range(1, 4):
                emit_pv_iq(0, iq, list(range(LA)), after_group=ilb)
            emit_wo(0, inject=inject)
            # consume preloaded fresh (t>=LA) v tiles first
            for iq in range(4):
                emit_pv_iq(1, iq, list(range(LA, LB)) + list(range(LA)))
            emit_wo(1)

    nc.compile()
    return nc


def _get_prog(which, *flags):
    key = (which,) + flags
    if key not in _PROG_CACHE:
        _PROG_CACHE[key] = _build_l1(*flags) if which == 1 else _build_l2(*flags)
    return _PROG_CACHE[key]


# core -> (batch, chunkA, chunkB): balanced causal pairing
_ASSIGN = [(b, j, 7 - j) for b in range(B) for j in range(4)]


def kernel(x, Wv, bv, Wg, bg, Wi, bi, gamma_q, beta_q, gamma_k, beta_k, Wo, bo):
    x = np.asarray(x, FP32)
    Wv = np.asarray(Wv, FP32); bv = np.asarray(bv, FP32)
    Wg = np.asarray(Wg, FP32); bg = np.asarray(bg, FP32)
    Wi = np.asarray(Wi, FP32); bi = np.asarray(bi, FP32)
    gamma_q = np.asarray(gamma_q, FP32); beta_q = np.asarray(beta_q, FP32)
    gamma_k = np.asarray(gamma_k, FP32); beta_k = np.asarray(beta_k, FP32)
    Wo = np.asarray(Wo, FP32); bo = np.asarray(bo, FP32)

    has_bv = bool(np.any(bv)); has_bg = bool(np.any(bg)); has_bo = bool(np.any(bo))
    nc1 = _get_prog(1, has_bv, has_bg)
    nc2 = _get_prog(2, has_bo)

    Wv_b = Wv.astype(BF16); Wg_b = Wg.astype(BF16)
    # Wi packed [k_lo, e_tile*H + h] for a single L1 DMA
    Wi_b = np.ascontiguousarray(
        Wi.astype(BF16).reshape(ET, 128, H).transpose(1, 0, 2).reshape(128, ET * H))
    # Wo packed [i_pair, k_lo, half, e] for DoubleRow-style paired loads
    Wo_b = np.ascontiguousarray(
        Wo.astype(BF16).reshape(IT // 2, 2, 128, E).transpose(0, 2, 1, 3))
    gq = (gamma_q * (S_SCALE / SCALE)).astype(FP32).reshape(H, 1)
    bq = (beta_q * (S_SCALE / SCALE)).astype(FP32).reshape(H, 1)
    gk = gamma_k.reshape(H, 1).copy(); bk = beta_k.reshape(H, 1).copy()
    bi_c = bi.reshape(H, 1).copy()
    bvr = bv.reshape(1, I).astype(BF16); bgr = bg.reshape(1, I).astype(BF16)
    bor = bo.reshape(1, E).astype(BF16)

    xg = x.reshape(B, N // CH, CH, E)
    in1 = []
    for (b, cA, cB) in _ASSIGN:
        xc = np.concatenate([xg[b, cA], xg[b, cB]], axis=0)  # [RPC, E]
        in1.append({
            "xT": np.ascontiguousarray(xc.T).astype(BF16),
            "Wv": Wv_b, "Wg": Wg_b, "Wi": Wi_b,
            "gq": gq, "bq": bq, "gk": gk, "bk": bk, "bi": bi_c,
            "bvr": bvr, "bgr": bgr,
        })
    res1 = run_bass_kernel_spmd(nc1, in1, core_ids=list(range(NC)))

    vfull = np.zeros((B, N // CH, CH, I), E4M3)
    kTfull = np.zeros((B, H, N), BF16)
    for c, (b, cA, cB) in enumerate(_ASSIGN):
        r = res1.results[c]
        v8 = (r["v_out"].astype(FP32) * V_SCALE).astype(E4M3)
        vfull[b, cA] = v8[:CH]
        vfull[b, cB] = v8[CH:]
        kTfull[b][:, cA * CH:(cA + 1) * CH] = r["kT_out"][:, :CH]
        kTfull[b][:, cB * CH:(cB + 1) * CH] = r["kT_out"][:, CH:]

    # extended causal mask: Mext[p, u] = 1 iff u >= p + 384
    Mext = (np.arange(896)[None, :] >= (np.arange(128)[:, None] + 384)).astype(FP32)

    in2 = []
    for c, (b, cA, cB) in enumerate(_ASSIGN):
        r = res1.results[c]
        kvk = np.zeros((NSTEPS, H, CH), BF16)
        kvv = np.zeros((NSTEPS, CH, I), E4M3)
        for base, cq in ((0, cA), (LA, cB)):
            kvk[base] = kTfull[b][:, cq * CH:(cq + 1) * CH]
            kvv[base] = vfull[b, cq]
            for idx in range(cq):
                kvk[base + 1 + idx] = kTfull[b][:, idx * CH:(idx + 1) * CH]
                kvv[base + 1 + idx] = vfull[b, idx]
        # [t, pair, kv_lo, kv_half, i] for DoubleRow lhsT tiles
        kvv2 = np.ascontiguousarray(
            kvv.reshape(NSTEPS, 2, 2, 128, I).transpose(0, 1, 3, 2, 4))
        gT_s = (r["gT_out"].astype(FP32) * (1.0 / OG_SCALE)).astype(BF16)
        gT_p = np.ascontiguousarray(
            gT_s.reshape(IT // 2, 2, 128, RPC).transpose(0, 2, 1, 3))
        in2.append({
            "qT": r["qT_out"], "gT": gT_p, "Wo": Wo_b,
            "kv_k": kvk, "kv_v": kvv2, "Mext": Mext, "bor": bor,
        })
    res2 = run_bass_kernel_spmd(nc2, in2, core_ids=list(range(NC)))

    out = np.zeros((B, N // CH, CH, E), FP32)
    for c, (b, cA, cB) in enumerate(_ASSIGN):
        yy = res2.results[c]["y"]
        out[b, cA] = yy[:CH]
        out[b, cB] = yy[CH:]
    return out.reshape(B, N, E)

